# revision 26
# baseline (speedup 1.0000x reference)
"""Cross-attention kernel for Trainium2, 8-core tensor-parallel over heads.

Problem (fixed shapes, fp32):
    patch_embed [2, 2048, 1024], pixel_embed [2, 2048, 1024]
    Wq/Wk/Wv [1024, 1024], Wo [1024, 1024], bo [1024]
    16 heads x 64 dim_head, softmax cross-attention, out [2, 2048, 1024].

Sharding: core c handles batch b = c // 4 and head-group g = c % 4
(4 heads = 256 inner cols). Each core computes a partial output
(its heads' contribution to out @ Wo) in bf16; host sums the 4 partials
per batch and adds the bias.

Per-core device program (all matmuls bf16 with fp32 PSUM accumulation):
    inputs shipped bf16, loaded natural [seq, d] in 512-row chunks (big
    contiguous DMA), transposed on the PE (128x128 identity-matmul tiles)
    kT = Wk_g^T @ pixelT      [256, m]
    v  = pixelT^T @ Wv_g (+ ones col)  [m, 4, 65]
    qT = Wq_g^T @ patchT      [256, n]
    per (pair, 1024-query block, key tile, head):
        sT = kT_h^T @ qT_h    (K=64, 2x512 free, row-group packed)
        eT = exp(scale * sT)  (ACT, [128, 1024] psum->sbuf bf16)
        oT[65] += v_aug^T @ eT  (row 64 accumulates Z)
    oT_n = oT[0:64] * (1/Z)   (DVE recip + gpsimd bcast + DVE mul, bf16)
    y_partial = oT_n^T @ Wo_g [n, 1024]  (bf16) -> DRAM

Emission is interleaved (single tile/PSUM pool scope) so attention on
query block 0 streams while pixel chunks 1-3 are still being loaded and
projected; patch chunks 2-3 and the qb0 output projection slot into the
ACT-bound stretches of later attention groups.
"""

import numpy as np

HEADS = 16
DH = 64
B = 2
N = 2048          # query seq len
M = 2048          # key seq len
D = 1024
N_CORES = 8
HPC = 4           # heads per core
C = HPC * DH      # 256 inner cols per core
SCALE = DH ** -0.5
P = 128
FREE = 512        # fp32 matmul moving free dim (one PSUM bank)
KT_D = D // P     # 8 contraction tiles for projections
NCH = N // FREE   # 4 input chunks of 512 rows
JT = M // P       # 16 key tiles
QB = 1024         # query block for attention/output phases

_cache = {}


def _build_nc():
    import concourse.bacc as bacc
    import concourse.mybir as mybir
    import concourse.tile as tile
    from concourse.masks import make_identity

    F32 = mybir.dt.float32
    F32R = mybir.dt.float32r
    BF16 = mybir.dt.bfloat16
    EXP = mybir.ActivationFunctionType.Exp

    nc = bacc.Bacc("TRN2", target_bir_lowering=False, debug=False,
                   num_devices=N_CORES)

    pe = nc.dram_tensor("pe", [N, D], BF16, kind="ExternalInput")
    xe = nc.dram_tensor("xe", [M, D], BF16, kind="ExternalInput")
    wq = nc.dram_tensor("wq", [D, C], BF16, kind="ExternalInput")
    wk = nc.dram_tensor("wk", [D, C], BF16, kind="ExternalInput")
    wv = nc.dram_tensor("wv", [D, C], BF16, kind="ExternalInput")
    wo = nc.dram_tensor("wo", [C, D], BF16, kind="ExternalInput")
    yp = nc.dram_tensor("yp", [N, D], BF16, kind="ExternalOutput")

    # chunked views: 512 seq rows -> [128 part, 4 subtiles, 1024]
    pe_c = pe.ap().rearrange("(ch s p) d -> ch p s d", p=P, s=4)
    xe_c = xe.ap().rearrange("(ch s p) d -> ch p s d", p=P, s=4)
    wq_t = wq.ap().rearrange("(ko ki) c -> ki ko c", ki=P)   # [128,8,256]
    wk_t = wk.ap().rearrange("(ko ki) c -> ki ko c", ki=P)
    wv_t = wv.ap().rearrange("(ko ki) c -> ki ko c", ki=P)
    wo_t = wo.ap().rearrange("(ko ki) n -> ki ko n", ki=P)   # [128,2,1024]
    yp_c = yp.ap().rearrange("(qc qt p) d -> qc p qt d", p=P, qt=8)

    with tile.TileContext(nc) as tc:
        with (
            tc.tile_pool(name="const", bufs=1) as const,
            tc.tile_pool(name="wpool", bufs=1) as wpool,
            tc.tile_pool(name="natp", bufs=3) as natp,
            tc.tile_pool(name="xTp", bufs=2) as xTp,
            tc.tile_pool(name="eT", bufs=6) as epool,
            tc.tile_pool(name="rzp", bufs=2) as rzp,
            tc.tile_pool(name="rzbp", bufs=2) as rzbp,
            tc.tile_pool(name="yst", bufs=2) as yst,
            tc.tile_pool(name="mm", bufs=2, space="PSUM") as mmp,
            tc.tile_pool(name="pacc", bufs=2, space="PSUM") as pacc,
        ):
            ident = const.tile([P, P], BF16)
            make_identity(nc, ident)

            wk_r = wpool.tile([P, KT_D, C], BF16, name="wk_r")
            wv_r = wpool.tile([P, KT_D, C], BF16, name="wv_r")
            wq_r = wpool.tile([P, KT_D, C], BF16, name="wq_r")
            wo_r = wpool.tile([P, 2, D], BF16, name="wo_r")

            # per-512-chunk tiles so attention deps are chunk-granular
            kTc = [wpool.tile([P, 2, FREE], BF16, name=f"kTc{i}")
                   for i in range(NCH)]
            qTc = [wpool.tile([P, 2, FREE], BF16, name=f"qTc{i}")
                   for i in range(NCH)]
            oTq = [wpool.tile([P, 2, QB], BF16, name=f"oTq{i}")
                   for i in range(N // QB)]
            vc = [wpool.tile([P, 4, HPC, DH + 1], BF16, name=f"vc{i}")
                  for i in range(NCH)]
            for i in range(NCH):
                nc.vector.memset(vc[i][:, :, :, DH], 1.0)

            # ---------------- front-end helpers --------------------------
            def load_w(dram_ap, shape, dst):
                stage = natp.tile([P, 4, D], BF16, tag="nat", name="wstg")
                st = stage[:].rearrange("p s d -> p (s d)")[
                    :, 0:int(np.prod(shape[1:]))].rearrange(
                    "p (a b) -> p a b", a=shape[1])
                nc.sync.dma_start(out=st, in_=dram_ap)
                nc.vector.tensor_copy(dst[:], st)

            def trans_chunk(src_c, ch):
                nat = natp.tile([P, 4, D], BF16, tag="nat", name="nat")
                nc.sync.dma_start(out=nat[:], in_=src_c[ch])
                xT = xTp.tile([P, KT_D, FREE], BF16, tag="xT", name="xT")
                for k2 in range(KT_D // 2):
                    pt = mmp.tile([P, 2 * FREE], BF16, tag="mm", name="pt")
                    for k in range(2):
                        kt = k2 * 2 + k
                        for s in range(4):
                            nc.tensor.transpose(
                                pt[:, k * FREE + s * P:
                                   k * FREE + (s + 1) * P],
                                nat[:, s, kt * P:(kt + 1) * P], ident[:])
                    nc.vector.tensor_copy(
                        xT[:, k2 * 2:(k2 + 1) * 2, :],
                        pt[:].rearrange("p (k q) -> p k q", k=2))
                return xT

            def proj_kq(w_r, dstT, xT):
                pq = mmp.tile([P, 2 * FREE], F32, tag="mm", name="pq")
                for mt in range(2):
                    for kt in range(KT_D):
                        nc.tensor.matmul(
                            pq[:, mt * FREE:(mt + 1) * FREE],
                            w_r[:, kt, mt * P:(mt + 1) * P],
                            xT[:, kt, :],
                            start=(kt == 0), stop=(kt == KT_D - 1))
                nc.vector.tensor_copy(
                    dstT[:], pq[:].rearrange("p (mt q) -> p mt q", mt=2))

            def proj_v(ch, xT):
                pv = mmp.tile([P, 4 * C], F32, tag="mm", name="pv")
                for s in range(4):
                    for kt in range(KT_D):
                        nc.tensor.matmul(
                            pv[:, s * C:(s + 1) * C],
                            xT[:, kt, s * P:(s + 1) * P],
                            wv_r[:, kt, :],
                            start=(kt == 0), stop=(kt == KT_D - 1))
                nc.vector.tensor_copy(
                    vc[ch][:, :, :, 0:DH],
                    pv[:].rearrange("p (s h e) -> p s h e", s=4, h=HPC))

            # ---------------- attention helpers ---------------------------
            def attn_group(qb, pair, po, jts):
                for jt in jts:
                    kch, jl = divmod(jt, 4)
                    for hh in range(2):
                        h = pair * 2 + hh
                        pst = mmp.tile([P, QB], F32, tag="mm", name="pst")
                        for qh in range(QB // FREE):
                            qch = qb * 2 + qh
                            nc.tensor.matmul(
                                pst[:, qh * FREE:(qh + 1) * FREE],
                                kTc[kch][hh * DH:(hh + 1) * DH, pair,
                                         jl * P:(jl + 1) * P],
                                qTc[qch][hh * DH:(hh + 1) * DH, pair, :],
                                start=True, stop=True,
                                tile_position=(hh * DH, 0))
                        eT = epool.tile([P, QB], BF16, tag="eT")
                        nc.scalar.activation(eT[:], pst[:], EXP,
                                             scale=SCALE)
                        for qh in range(QB // FREE):
                            nc.tensor.matmul(
                                po[hh][:, qh * FREE:(qh + 1) * FREE],
                                vc[kch][:, jl, h, :],
                                eT[:, qh * FREE:(qh + 1) * FREE],
                                start=(jt == 0), stop=(jt == JT - 1))

            def attn_norm(qb, pair, po, qhs=(0, 1)):
                # copy PSUM accumulators to SBUF first so the po banks
                # free up for the next pair; normalize from the copies.
                for hh in range(2):
                    zc = rzp.tile([DH + 1, QB], F32, tag="zc",
                                  name=f"zc{qb}{pair}{hh}")
                    nc.vector.tensor_copy(zc[:], po[hh][:])
                    for qh in qhs:
                        sl = slice(qh * FREE, (qh + 1) * FREE)
                        rz = rzp.tile([1, FREE], F32, tag="rz")
                        nc.vector.reciprocal(rz[:], zc[DH:DH + 1, sl])
                        rzb = rzbp.tile([DH, FREE], F32, tag="rzb")
                        nc.gpsimd.partition_broadcast(rzb[:], rz[:])
                        nc.vector.tensor_mul(
                            oTq[qb][hh * DH:(hh + 1) * DH, pair, sl],
                            zc[0:DH, sl], rzb[:])

            def new_po():
                return [pacc.tile([DH + 1, QB], F32, tag="po",
                                  name=f"po{hh}") for hh in range(2)]

            def out_proj_tiles(qb, ys, qrange):
                for qt8 in qrange:
                    py = mmp.tile([P, D], F32, tag="mm", name="py")
                    for nk in range(D // FREE):
                        for ct in range(2):
                            nc.tensor.matmul(
                                py[:, nk * FREE:(nk + 1) * FREE],
                                oTq[qb][:, ct, qt8 * P:(qt8 + 1) * P],
                                wo_r[:, ct, nk * FREE:(nk + 1) * FREE],
                                start=(ct == 0), stop=(ct == 1))
                    nc.vector.tensor_copy(ys[:, qt8, :], py[:])

            # ======================= emission =============================
            # patch chunks 0-1 (query block 0), weights
            paT0 = trans_chunk(pe_c, 0)
            load_w(wq_t, [P, KT_D, C], wq_r)
            proj_kq(wq_r, qTc[0], paT0)
            paT1 = trans_chunk(pe_c, 1)
            proj_kq(wq_r, qTc[1], paT1)
            load_w(wk_t, [P, KT_D, C], wk_r)
            load_w(wv_t, [P, KT_D, C], wv_r)

            # pixel chunks interleaved with attention qb0/pair0
            po00 = new_po()
            for ch in range(NCH):
                xT = trans_chunk(xe_c, ch)
                proj_kq(wk_r, kTc[ch], xT)
                proj_v(ch, xT)
                attn_group(0, 0, po00, range(4 * ch, 4 * ch + 4))
            attn_norm(0, 0, po00)

            # qb0/pair1 with patch chunks 2-3 and wo slotted in
            po01 = new_po()
            attn_group(0, 1, po01, range(0, 4))
            paT2 = trans_chunk(pe_c, 2)
            proj_kq(wq_r, qTc[2], paT2)
            attn_group(0, 1, po01, range(4, 8))
            paT3 = trans_chunk(pe_c, 3)
            proj_kq(wq_r, qTc[3], paT3)
            load_w(wo_t, [P, 2, D], wo_r)
            attn_group(0, 1, po01, range(8, 16))
            attn_norm(0, 1, po01)

            # qb1/pair0 with qb0 output projection slotted in (late enough
            # that the qb0/pair1 normalization chain has finished)
            ys0 = yst.tile([P, 8, D], BF16, tag="ys", name="ys0")
            po10 = new_po()
            attn_group(1, 0, po10, range(0, 8))
            out_proj_tiles(0, ys0, range(0, 4))
            attn_group(1, 0, po10, range(8, 12))
            out_proj_tiles(0, ys0, range(4, 8))
            attn_group(1, 0, po10, range(12, 16))
            attn_norm(1, 0, po10)
            nc.sync.dma_start(out=yp_c[0], in_=ys0[:])

            # qb1/pair1, then qb1 output projection; the tail normalization
            # is split by query half so the projection overlaps it
            po11 = new_po()
            attn_group(1, 1, po11, range(0, 16))
            zcs = []
            for hh in range(2):
                zc = rzp.tile([DH + 1, QB], F32, tag="zc", name=f"zct{hh}")
                nc.vector.tensor_copy(zc[:], po11[hh][:])
                zcs.append(zc)
            ys1 = yst.tile([P, 8, D], BF16, tag="ys", name="ys1")
            for qh in range(2):
                sl = slice(qh * FREE, (qh + 1) * FREE)
                for hh in range(2):
                    rz = rzp.tile([1, FREE], F32, tag="rz")
                    nc.vector.reciprocal(rz[:], zcs[hh][DH:DH + 1, sl])
                    rzb = rzbp.tile([DH, FREE], F32, tag="rzb")
                    nc.gpsimd.partition_broadcast(rzb[:], rz[:])
                    nc.vector.tensor_mul(
                        oTq[1][hh * DH:(hh + 1) * DH, 1, sl],
                        zcs[hh][0:DH, sl], rzb[:])
                out_proj_tiles(1, ys1, range(qh * 4, qh * 4 + 4))
            nc.sync.dma_start(out=yp_c[1], in_=ys1[:])

    nc.compile()
    return nc


def get_nc():
    if "nc" not in _cache:
        _cache["nc"] = _build_nc()
    return _cache["nc"]


def make_core_inputs(patch_embed, pixel_embed, Wq, Wk, Wv, Wo, c):
    import ml_dtypes

    bf16 = ml_dtypes.bfloat16
    b, g = divmod(c, HPC)
    sl = slice(g * C, (g + 1) * C)
    return {
        "pe": np.ascontiguousarray(np.asarray(patch_embed[b]).astype(bf16)),
        "xe": np.ascontiguousarray(np.asarray(pixel_embed[b]).astype(bf16)),
        "wq": np.ascontiguousarray(np.asarray(Wq[:, sl]).astype(bf16)),
        "wk": np.ascontiguousarray(np.asarray(Wk[:, sl]).astype(bf16)),
        "wv": np.ascontiguousarray(np.asarray(Wv[:, sl]).astype(bf16)),
        "wo": np.ascontiguousarray(np.asarray(Wo[sl, :]).astype(bf16)),
    }


def kernel(patch_embed, pixel_embed, Wq, Wk, Wv, Wo, bo):
    from concourse.bass_utils import run_bass_kernel_spmd

    nc = get_nc()
    in_maps = [make_core_inputs(patch_embed, pixel_embed, Wq, Wk, Wv, Wo, c)
               for c in range(N_CORES)]
    res = run_bass_kernel_spmd(nc, in_maps, core_ids=list(range(N_CORES)))
    out = np.empty((B, N, D), dtype=np.float32)
    for b in range(B):
        acc = res.results[b * HPC + 0]["yp"].astype(np.float32)
        for g in range(1, HPC):
            acc = acc + res.results[b * HPC + g]["yp"].astype(np.float32)
        out[b] = acc + np.asarray(bo, dtype=np.float32)[None, :]
    return out


# revision 27
# speedup vs baseline: 1.0102x; 1.0102x over previous
"""Cross-attention kernel for Trainium2, 8-core tensor-parallel over heads.

Problem (fixed shapes, fp32):
    patch_embed [2, 2048, 1024], pixel_embed [2, 2048, 1024]
    Wq/Wk/Wv [1024, 1024], Wo [1024, 1024], bo [1024]
    16 heads x 64 dim_head, softmax cross-attention, out [2, 2048, 1024].

Sharding: core c handles batch b = c // 4 and head-group g = c % 4
(4 heads = 256 inner cols). Each core computes a partial output
(its heads' contribution to out @ Wo) in bf16; host sums the 4 partials
per batch and adds the bias.

Per-core device program (all matmuls bf16 with fp32 PSUM accumulation):
    inputs shipped bf16, loaded natural [seq, d] in 512-row chunks (big
    contiguous DMA), transposed on the PE (128x128 identity-matmul tiles)
    kT = Wk_g^T @ pixelT      [256, m]
    v  = pixelT^T @ Wv_g (+ ones col)  [m, 4, 65]
    qT = Wq_g^T @ patchT      [256, n]
    per (pair, 1024-query block, key tile, head):
        sT = kT_h^T @ qT_h    (K=64, 2x512 free, row-group packed)
        eT = exp(scale * sT)  (ACT, [128, 1024] psum->sbuf bf16)
        oT[65] += v_aug^T @ eT  (row 64 accumulates Z)
    oT_n = oT[0:64] * (1/Z)   (DVE recip + gpsimd bcast + DVE mul, bf16)
    y_partial = oT_n^T @ Wo_g [n, 1024]  (bf16) -> DRAM

Emission is interleaved (single tile/PSUM pool scope) so attention on
query block 0 streams while pixel chunks 1-3 are still being loaded and
projected; patch chunks 2-3 and the qb0 output projection slot into the
ACT-bound stretches of later attention groups.
"""

import numpy as np

HEADS = 16
DH = 64
B = 2
N = 2048          # query seq len
M = 2048          # key seq len
D = 1024
N_CORES = 8
HPC = 4           # heads per core
C = HPC * DH      # 256 inner cols per core
SCALE = DH ** -0.5
P = 128
FREE = 512        # fp32 matmul moving free dim (one PSUM bank)
KT_D = D // P     # 8 contraction tiles for projections
NCH = N // FREE   # 4 input chunks of 512 rows
JT = M // P       # 16 key tiles
QB = 1024         # query block for attention/output phases

_cache = {}


def _build_nc():
    import concourse.bacc as bacc
    import concourse.mybir as mybir
    import concourse.tile as tile
    from concourse.masks import make_identity

    F32 = mybir.dt.float32
    F32R = mybir.dt.float32r
    BF16 = mybir.dt.bfloat16
    EXP = mybir.ActivationFunctionType.Exp

    nc = bacc.Bacc("TRN2", target_bir_lowering=False, debug=False,
                   num_devices=N_CORES)

    pe = nc.dram_tensor("pe", [N, D], BF16, kind="ExternalInput")
    xe = nc.dram_tensor("xe", [M, D], BF16, kind="ExternalInput")
    wq = nc.dram_tensor("wq", [D, C], BF16, kind="ExternalInput")
    wk = nc.dram_tensor("wk", [D, C], BF16, kind="ExternalInput")
    wv = nc.dram_tensor("wv", [D, C], BF16, kind="ExternalInput")
    wo = nc.dram_tensor("wo", [C, D], BF16, kind="ExternalInput")
    yp = nc.dram_tensor("yp", [N, D], BF16, kind="ExternalOutput")

    # chunked views: 512 seq rows -> [128 part, 4 subtiles, 1024]
    pe_c = pe.ap().rearrange("(ch s p) d -> ch p s d", p=P, s=4)
    xe_c = xe.ap().rearrange("(ch s p) d -> ch p s d", p=P, s=4)
    wq_t = wq.ap().rearrange("(ko ki) c -> ki ko c", ki=P)   # [128,8,256]
    wk_t = wk.ap().rearrange("(ko ki) c -> ki ko c", ki=P)
    wv_t = wv.ap().rearrange("(ko ki) c -> ki ko c", ki=P)
    wo_t = wo.ap().rearrange("(ko ki) n -> ki ko n", ki=P)   # [128,2,1024]
    yp_c = yp.ap().rearrange("(qc qt p) d -> qc p qt d", p=P, qt=8)

    with tile.TileContext(nc) as tc:
        with (
            tc.tile_pool(name="const", bufs=1) as const,
            tc.tile_pool(name="wpool", bufs=1) as wpool,
            tc.tile_pool(name="natp", bufs=4) as natp,
            tc.tile_pool(name="xTp", bufs=2) as xTp,
            tc.tile_pool(name="eT", bufs=6) as epool,
            tc.tile_pool(name="rzp", bufs=2) as rzp,
            tc.tile_pool(name="rzbp", bufs=2) as rzbp,
            tc.tile_pool(name="yst", bufs=2) as yst,
            tc.tile_pool(name="mm", bufs=2, space="PSUM") as mmp,
            tc.tile_pool(name="pacc", bufs=2, space="PSUM") as pacc,
        ):
            ident = const.tile([P, P], BF16)
            make_identity(nc, ident)

            wk_r = wpool.tile([P, KT_D, C], BF16, name="wk_r")
            wv_r = wpool.tile([P, KT_D, C], BF16, name="wv_r")
            wq_r = wpool.tile([P, KT_D, C], BF16, name="wq_r")
            wo_r = wpool.tile([P, 2, D], BF16, name="wo_r")

            # per-512-chunk tiles so attention deps are chunk-granular
            kTc = [wpool.tile([P, 2, FREE], BF16, name=f"kTc{i}")
                   for i in range(NCH)]
            qTc = [wpool.tile([P, 2, FREE], BF16, name=f"qTc{i}")
                   for i in range(NCH)]
            oTq = [wpool.tile([P, 2, QB], BF16, name=f"oTq{i}")
                   for i in range(N // QB)]
            vc = [wpool.tile([P, 4, HPC, DH + 1], BF16, name=f"vc{i}")
                  for i in range(NCH)]
            for i in range(NCH):
                nc.vector.memset(vc[i][:, :, :, DH], 1.0)

            # ---------------- front-end helpers --------------------------
            def load_w(dram_ap, shape, dst):
                stage = natp.tile([P, 4, D], BF16, tag="nat", name="wstg")
                st = stage[:].rearrange("p s d -> p (s d)")[
                    :, 0:int(np.prod(shape[1:]))].rearrange(
                    "p (a b) -> p a b", a=shape[1])
                nc.sync.dma_start(out=st, in_=dram_ap)
                nc.vector.tensor_copy(dst[:], st)

            def trans_chunk(src_c, ch):
                nat = natp.tile([P, 4, D], BF16, tag="nat", name="nat")
                nc.sync.dma_start(out=nat[:], in_=src_c[ch])
                xT = xTp.tile([P, KT_D, FREE], BF16, tag="xT", name="xT")
                for k2 in range(KT_D // 2):
                    pt = mmp.tile([P, 2 * FREE], BF16, tag="mm", name="pt")
                    for k in range(2):
                        kt = k2 * 2 + k
                        for s in range(4):
                            nc.tensor.transpose(
                                pt[:, k * FREE + s * P:
                                   k * FREE + (s + 1) * P],
                                nat[:, s, kt * P:(kt + 1) * P], ident[:])
                    nc.vector.tensor_copy(
                        xT[:, k2 * 2:(k2 + 1) * 2, :],
                        pt[:].rearrange("p (k q) -> p k q", k=2))
                return xT

            def proj_kq(w_r, dstT, xT):
                pq = mmp.tile([P, 2 * FREE], F32, tag="mm", name="pq")
                for mt in range(2):
                    for kt in range(KT_D):
                        nc.tensor.matmul(
                            pq[:, mt * FREE:(mt + 1) * FREE],
                            w_r[:, kt, mt * P:(mt + 1) * P],
                            xT[:, kt, :],
                            start=(kt == 0), stop=(kt == KT_D - 1))
                nc.vector.tensor_copy(
                    dstT[:], pq[:].rearrange("p (mt q) -> p mt q", mt=2))

            def proj_v(ch, xT):
                pv = mmp.tile([P, 4 * C], F32, tag="mm", name="pv")
                for s in range(4):
                    for kt in range(KT_D):
                        nc.tensor.matmul(
                            pv[:, s * C:(s + 1) * C],
                            xT[:, kt, s * P:(s + 1) * P],
                            wv_r[:, kt, :],
                            start=(kt == 0), stop=(kt == KT_D - 1))
                nc.vector.tensor_copy(
                    vc[ch][:, :, :, 0:DH],
                    pv[:].rearrange("p (s h e) -> p s h e", s=4, h=HPC))

            # ---------------- attention helpers ---------------------------
            def attn_group(qb, pair, po, jts):
                for jt in jts:
                    kch, jl = divmod(jt, 4)
                    for hh in range(2):
                        h = pair * 2 + hh
                        pst = mmp.tile([P, QB], F32, tag="mm", name="pst")
                        for qh in range(QB // FREE):
                            qch = qb * 2 + qh
                            nc.tensor.matmul(
                                pst[:, qh * FREE:(qh + 1) * FREE],
                                kTc[kch][hh * DH:(hh + 1) * DH, pair,
                                         jl * P:(jl + 1) * P],
                                qTc[qch][hh * DH:(hh + 1) * DH, pair, :],
                                start=True, stop=True,
                                tile_position=(hh * DH, 0))
                        eT = epool.tile([P, QB], BF16, tag="eT")
                        nc.scalar.activation(eT[:], pst[:], EXP,
                                             scale=SCALE)
                        for qh in range(QB // FREE):
                            nc.tensor.matmul(
                                po[hh][:, qh * FREE:(qh + 1) * FREE],
                                vc[kch][:, jl, h, :],
                                eT[:, qh * FREE:(qh + 1) * FREE],
                                start=(jt == 0), stop=(jt == JT - 1))

            def attn_norm(qb, pair, po, qhs=(0, 1)):
                # copy PSUM accumulators to SBUF first so the po banks
                # free up for the next pair; normalize from the copies.
                for hh in range(2):
                    zc = rzp.tile([DH + 1, QB], F32, tag="zc",
                                  name=f"zc{qb}{pair}{hh}")
                    nc.vector.tensor_copy(zc[:], po[hh][:])
                    for qh in qhs:
                        sl = slice(qh * FREE, (qh + 1) * FREE)
                        rz = rzp.tile([1, FREE], F32, tag="rz")
                        nc.vector.reciprocal(rz[:], zc[DH:DH + 1, sl])
                        rzb = rzbp.tile([DH, FREE], F32, tag="rzb")
                        nc.gpsimd.partition_broadcast(rzb[:], rz[:])
                        nc.vector.tensor_mul(
                            oTq[qb][hh * DH:(hh + 1) * DH, pair, sl],
                            zc[0:DH, sl], rzb[:])

            def new_po():
                return [pacc.tile([DH + 1, QB], F32, tag="po",
                                  name=f"po{hh}") for hh in range(2)]

            def out_proj_tiles(qb, ys, qrange):
                for qt8 in qrange:
                    py = mmp.tile([P, D], F32, tag="mm", name="py")
                    for nk in range(D // FREE):
                        for ct in range(2):
                            nc.tensor.matmul(
                                py[:, nk * FREE:(nk + 1) * FREE],
                                oTq[qb][:, ct, qt8 * P:(qt8 + 1) * P],
                                wo_r[:, ct, nk * FREE:(nk + 1) * FREE],
                                start=(ct == 0), stop=(ct == 1))
                    nc.vector.tensor_copy(ys[:, qt8, :], py[:])

            # ======================= emission =============================
            # patch chunks 0-1 (query block 0), weights
            paT0 = trans_chunk(pe_c, 0)
            load_w(wq_t, [P, KT_D, C], wq_r)
            proj_kq(wq_r, qTc[0], paT0)
            paT1 = trans_chunk(pe_c, 1)
            proj_kq(wq_r, qTc[1], paT1)
            load_w(wk_t, [P, KT_D, C], wk_r)
            load_w(wv_t, [P, KT_D, C], wv_r)

            # pixel chunks interleaved with attention qb0/pair0
            po00 = new_po()
            for ch in range(NCH):
                xT = trans_chunk(xe_c, ch)
                proj_kq(wk_r, kTc[ch], xT)
                proj_v(ch, xT)
                attn_group(0, 0, po00, range(4 * ch, 4 * ch + 4))
            attn_norm(0, 0, po00)

            # qb0/pair1 with patch chunks 2-3 and wo slotted in
            po01 = new_po()
            attn_group(0, 1, po01, range(0, 4))
            paT2 = trans_chunk(pe_c, 2)
            proj_kq(wq_r, qTc[2], paT2)
            attn_group(0, 1, po01, range(4, 8))
            paT3 = trans_chunk(pe_c, 3)
            proj_kq(wq_r, qTc[3], paT3)
            load_w(wo_t, [P, 2, D], wo_r)
            attn_group(0, 1, po01, range(8, 16))
            attn_norm(0, 1, po01)

            # qb1/pair0 with qb0 output projection slotted in (late enough
            # that the qb0/pair1 normalization chain has finished)
            ys0 = yst.tile([P, 8, D], BF16, tag="ys", name="ys0")
            po10 = new_po()
            attn_group(1, 0, po10, range(0, 8))
            out_proj_tiles(0, ys0, range(0, 4))
            attn_group(1, 0, po10, range(8, 12))
            out_proj_tiles(0, ys0, range(4, 8))
            attn_group(1, 0, po10, range(12, 16))
            attn_norm(1, 0, po10)
            nc.sync.dma_start(out=yp_c[0], in_=ys0[:])

            # qb1/pair1, then qb1 output projection; the tail normalization
            # is split by query half so the projection overlaps it
            po11 = new_po()
            attn_group(1, 1, po11, range(0, 16))
            zcs = []
            for hh in range(2):
                zc = rzp.tile([DH + 1, QB], F32, tag="zc", name=f"zct{hh}")
                nc.vector.tensor_copy(zc[:], po11[hh][:])
                zcs.append(zc)
            ys1 = yst.tile([P, 8, D], BF16, tag="ys", name="ys1")
            for qh in range(2):
                sl = slice(qh * FREE, (qh + 1) * FREE)
                for hh in range(2):
                    rz = rzp.tile([1, FREE], F32, tag="rz")
                    nc.vector.reciprocal(rz[:], zcs[hh][DH:DH + 1, sl])
                    rzb = rzbp.tile([DH, FREE], F32, tag="rzb")
                    nc.gpsimd.partition_broadcast(rzb[:], rz[:])
                    nc.vector.tensor_mul(
                        oTq[1][hh * DH:(hh + 1) * DH, 1, sl],
                        zcs[hh][0:DH, sl], rzb[:])
                out_proj_tiles(1, ys1, range(qh * 4, qh * 4 + 4))
            nc.sync.dma_start(out=yp_c[1], in_=ys1[:])

    nc.compile()
    return nc


def get_nc():
    if "nc" not in _cache:
        _cache["nc"] = _build_nc()
    return _cache["nc"]


def make_core_inputs(patch_embed, pixel_embed, Wq, Wk, Wv, Wo, c):
    import ml_dtypes

    bf16 = ml_dtypes.bfloat16
    b, g = divmod(c, HPC)
    sl = slice(g * C, (g + 1) * C)
    return {
        "pe": np.ascontiguousarray(np.asarray(patch_embed[b]).astype(bf16)),
        "xe": np.ascontiguousarray(np.asarray(pixel_embed[b]).astype(bf16)),
        "wq": np.ascontiguousarray(np.asarray(Wq[:, sl]).astype(bf16)),
        "wk": np.ascontiguousarray(np.asarray(Wk[:, sl]).astype(bf16)),
        "wv": np.ascontiguousarray(np.asarray(Wv[:, sl]).astype(bf16)),
        "wo": np.ascontiguousarray(np.asarray(Wo[sl, :]).astype(bf16)),
    }


def kernel(patch_embed, pixel_embed, Wq, Wk, Wv, Wo, bo):
    from concourse.bass_utils import run_bass_kernel_spmd

    nc = get_nc()
    in_maps = [make_core_inputs(patch_embed, pixel_embed, Wq, Wk, Wv, Wo, c)
               for c in range(N_CORES)]
    res = run_bass_kernel_spmd(nc, in_maps, core_ids=list(range(N_CORES)))
    out = np.empty((B, N, D), dtype=np.float32)
    for b in range(B):
        acc = res.results[b * HPC + 0]["yp"].astype(np.float32)
        for g in range(1, HPC):
            acc = acc + res.results[b * HPC + g]["yp"].astype(np.float32)
        out[b] = acc + np.asarray(bo, dtype=np.float32)[None, :]
    return out
